# revision 5
# baseline (speedup 1.0000x reference)
"""Trainium2 Bass kernel for MF embedding-lookup + dot-product scoring.

out[u, i] = dot(user_hiddens[user_ids[u]], item_hiddens[item_ids[i]])

Sharding: batch users split 8 ways (512/core); tables replicated to every
core's HBM. Each core gathers its 512 user rows + all 4096 item rows with
indirect DMA, PE-transposes them to [64, batch] layout, runs 32 matmuls
[64,128]^T @ [64,512] -> PSUM, copies to SBUF (ACT/DVE split) and DMAs its
[512, 4096] output slab. Host concatenates the 8 slabs along axis 0.
"""

import numpy as np

import concourse.bass as bass
import concourse.bacc as bacc
import concourse.mybir as mybir
import concourse.tile as tile
from concourse.bass_utils import run_bass_kernel_spmd
from concourse.masks import make_identity

NUM_USERS = 1_000_000
NUM_ITEMS = 100_000
D = 64
BU = 4096
BI = 4096
N_CORES = 8
UC = BU // N_CORES  # users per core = 512
P = 128
UT = UC // P        # user tiles per core = 4
IT = BI // P        # item tiles = 32
NBLK = 512          # matmul moving free dim
NT = BI // NBLK     # n blocks = 8
GCH = 8             # item-gather chunk, in tiles of 128 rows

_cache = {}


def _build():
    nc = bacc.Bacc()
    ut_dram = nc.dram_tensor(
        "user_table", [NUM_USERS, D], mybir.dt.float32, kind="ExternalInput"
    )
    it_dram = nc.dram_tensor(
        "item_table", [NUM_ITEMS, D], mybir.dt.float32, kind="ExternalInput"
    )
    uid_dram = nc.dram_tensor("uids", [P, UT], mybir.dt.int32, kind="ExternalInput")
    iid_dram = nc.dram_tensor("iids", [P, IT], mybir.dt.int32, kind="ExternalInput")
    out_dram = nc.dram_tensor(
        "out", [UC, BI], mybir.dt.float32, kind="ExternalOutput"
    )

    with tile.TileContext(nc) as tc:
        with (
            tc.tile_pool(name="const", bufs=1) as constp,
            tc.tile_pool(name="idx", bufs=1) as idxp,
            tc.tile_pool(name="gath", bufs=12) as gathp,
            tc.tile_pool(name="ops", bufs=1) as opsp,
            tc.tile_pool(name="tp", bufs=4, space="PSUM") as tpp,
            tc.tile_pool(name="mm", bufs=4, space="PSUM") as mmp,
            tc.tile_pool(name="outp", bufs=2) as outp,
        ):
            ident = constp.tile([P, P], mybir.dt.float32)
            make_identity(nc, ident[:])

            uids = idxp.tile([P, UT], mybir.dt.int32)
            iids = idxp.tile([P, IT], mybir.dt.int32)
            nc.sync.dma_start(out=uids[:], in_=uid_dram[:])
            nc.sync.dma_start(out=iids[:], in_=iid_dram[:])

            # Gather rows 128 at a time (one index per partition — the only
            # indirect-DMA shape that is correct on HW), then PE-transpose
            # each [128, 64] tile into [64, batch] layout.
            utr = opsp.tile([D, UC], mybir.dt.float32)
            vtr = opsp.tile([D, BI], mybir.dt.float32)
            for t in range(UT):
                g = gathp.tile([P, D], mybir.dt.float32)
                nc.gpsimd.indirect_dma_start(
                    out=g[:],
                    out_offset=None,
                    in_=ut_dram[:],
                    in_offset=bass.IndirectOffsetOnAxis(
                        ap=uids[:, t : t + 1], axis=0
                    ),
                )
                ps = tpp.tile([D, P], mybir.dt.float32)
                nc.tensor.transpose(ps[:], g[:], ident[:])
                nc.vector.tensor_copy(out=utr[:, t * P : (t + 1) * P], in_=ps[:])
            for t in range(IT):
                g = gathp.tile([P, D], mybir.dt.float32)
                nc.gpsimd.indirect_dma_start(
                    out=g[:],
                    out_offset=None,
                    in_=it_dram[:],
                    in_offset=bass.IndirectOffsetOnAxis(
                        ap=iids[:, t : t + 1], axis=0
                    ),
                )
                ps = tpp.tile([D, P], mybir.dt.float32)
                nc.tensor.transpose(ps[:], g[:], ident[:])
                if t % 2 == 0:
                    nc.vector.tensor_copy(out=vtr[:, t * P : (t + 1) * P], in_=ps[:])
                else:
                    nc.scalar.copy(out=vtr[:, t * P : (t + 1) * P], in_=ps[:])

            for mt in range(UT):
                ot = outp.tile([P, BI], mybir.dt.float32)
                for nt in range(NT):
                    ps = mmp.tile([P, NBLK], mybir.dt.float32)
                    nc.tensor.matmul(
                        ps[:],
                        lhsT=utr[:, mt * P : (mt + 1) * P],
                        rhs=vtr[:, nt * NBLK : (nt + 1) * NBLK],
                        start=True,
                        stop=True,
                    )
                    if nt % 2 == 0:
                        nc.scalar.copy(
                            out=ot[:, nt * NBLK : (nt + 1) * NBLK], in_=ps[:]
                        )
                    else:
                        nc.vector.tensor_copy(
                            out=ot[:, nt * NBLK : (nt + 1) * NBLK], in_=ps[:]
                        )
                nc.sync.dma_start(
                    out=out_dram[mt * P : (mt + 1) * P, :], in_=ot[:]
                )
    nc.finalize()
    return nc


def kernel(user_hiddens, item_hiddens, user_ids, item_ids, **_):
    user_hiddens = np.ascontiguousarray(user_hiddens, dtype=np.float32)
    item_hiddens = np.ascontiguousarray(item_hiddens, dtype=np.float32)
    user_ids = np.asarray(user_ids)
    item_ids = np.asarray(item_ids)

    if "nc" not in _cache:
        _cache["nc"] = _build()
    nc = _cache["nc"]

    # [P, T] transposed id layout: idx[p, t] = ids[t*128 + p]
    iids_t = np.ascontiguousarray(
        item_ids.astype(np.int32).reshape(IT, P).T
    )
    in_maps = []
    for c in range(N_CORES):
        uc = user_ids[c * UC : (c + 1) * UC]
        uids_t = np.ascontiguousarray(uc.astype(np.int32).reshape(UT, P).T)
        in_maps.append(
            {
                "user_table": user_hiddens,
                "item_table": item_hiddens,
                "uids": uids_t,
                "iids": iids_t,
            }
        )

    res = run_bass_kernel_spmd(nc, in_maps, list(range(N_CORES)))
    out = np.concatenate([res.results[c]["out"] for c in range(N_CORES)], axis=0)
    return out


# revision 6
# speedup vs baseline: 1.4558x; 1.4558x over previous
"""Trainium2 Bass kernel for MF embedding-lookup + dot-product scoring.

out[u, i] = dot(user_hiddens[user_ids[u]], item_hiddens[item_ids[i]])

Sharding: 2D over 8 cores — 4 user groups (1024 users) x 2 item groups
(2048 items); tables replicated to every core's HBM. Per core:
  - indirect-DMA gathers 128 rows/call (one index per partition), 8 user
    calls + 16 item calls
  - PE transpose to [64, batch]; split each f32 value into bf16 hi+lo
  - per item tile: 3-term bf16 matmuls (hi*hi + hi*lo + lo*hi) accumulate
    in f32 PSUM -> ~1e-5 rel err at ~4x the fp32 matmul speed
  - item tile stationary, users moving: the matmul for item tile t fires
    as soon as tile t's gather lands (no global barrier on the gathers)
  - output [2048 items, 1024 users] written in 512 KB contiguous chunks
Host transposes each core slab into the final [4096, 4096].
"""

import numpy as np

import concourse.bacc as bacc
import concourse.bass as bass
import concourse.mybir as mybir
import concourse.tile as tile
from concourse.bass_utils import run_bass_kernel_spmd
from concourse.masks import make_identity

NUM_USERS = 1_000_000
NUM_ITEMS = 100_000
D = 64
BU = 4096
BI = 4096
N_CORES = 8
RU = 4              # user groups
RI = 2              # item groups
UC = BU // RU       # users per core = 1024
IC = BI // RI       # items per core = 2048
P = 128
UT = UC // P        # user tiles per core = 8
IT = IC // P        # item tiles per core = 16
NBLK = 512          # matmul moving free dim
NH = UC // NBLK     # user halves per item tile = 2

_cache = {}


def _build():
    nc = bacc.Bacc()
    ut_dram = nc.dram_tensor(
        "user_table", [NUM_USERS, D], mybir.dt.float32, kind="ExternalInput"
    )
    it_dram = nc.dram_tensor(
        "item_table", [NUM_ITEMS, D], mybir.dt.float32, kind="ExternalInput"
    )
    uid_dram = nc.dram_tensor("uids", [P, UT], mybir.dt.int32, kind="ExternalInput")
    iid_dram = nc.dram_tensor("iids", [P, IT], mybir.dt.int32, kind="ExternalInput")
    out_dram = nc.dram_tensor(
        "out", [IC, UC], mybir.dt.float32, kind="ExternalOutput"
    )

    f32 = mybir.dt.float32
    bf16 = mybir.dt.bfloat16

    with tile.TileContext(nc) as tc:
        with (
            tc.tile_pool(name="const", bufs=1) as constp,
            tc.tile_pool(name="idx", bufs=1) as idxp,
            tc.tile_pool(name="gath", bufs=12) as gathp,
            tc.tile_pool(name="ops", bufs=1) as opsp,
            tc.tile_pool(name="vt", bufs=4) as vtp,
            tc.tile_pool(name="tp", bufs=2, space="PSUM") as tpp,
            tc.tile_pool(name="mm", bufs=4, space="PSUM") as mmp,
            tc.tile_pool(name="outp", bufs=3) as outp,
        ):
            ident = constp.tile([P, P], f32)
            make_identity(nc, ident[:])

            uids = idxp.tile([P, UT], mybir.dt.int32)
            iids = idxp.tile([P, IT], mybir.dt.int32)
            nc.sync.dma_start(out=uids[:], in_=uid_dram[:])
            nc.sync.dma_start(out=iids[:], in_=iid_dram[:])

            # --- user prologue: gather + transpose + bf16 hi/lo split ---
            uhi = opsp.tile([D, UC], bf16)
            ulo = opsp.tile([D, UC], bf16)
            for t in range(UT):
                g = gathp.tile([P, D], f32)
                nc.gpsimd.indirect_dma_start(
                    out=g[:],
                    out_offset=None,
                    in_=ut_dram[:],
                    in_offset=bass.IndirectOffsetOnAxis(
                        ap=uids[:, t : t + 1], axis=0
                    ),
                )
                ps = tpp.tile([D, P], f32)
                nc.tensor.transpose(ps[:], g[:], ident[:])
                sl = slice(t * P, (t + 1) * P)
                nc.scalar.copy(out=uhi[:, sl], in_=ps[:])
                nc.vector.tensor_tensor(
                    out=ulo[:, sl],
                    in0=ps[:],
                    in1=uhi[:, sl],
                    op=mybir.AluOpType.subtract,
                )

            # --- item stream: gather -> transpose -> hi/lo -> matmuls -> out ---
            for t in range(IT):
                g = gathp.tile([P, D], f32)
                nc.gpsimd.indirect_dma_start(
                    out=g[:],
                    out_offset=None,
                    in_=it_dram[:],
                    in_offset=bass.IndirectOffsetOnAxis(
                        ap=iids[:, t : t + 1], axis=0
                    ),
                )
                ps = tpp.tile([D, P], f32)
                nc.tensor.transpose(ps[:], g[:], ident[:])
                vhi = vtp.tile([D, P], bf16)
                vlo = vtp.tile([D, P], bf16)
                nc.scalar.copy(out=vhi[:], in_=ps[:])
                nc.vector.tensor_tensor(
                    out=vlo[:], in0=ps[:], in1=vhi[:], op=mybir.AluOpType.subtract
                )

                ot = outp.tile([P, UC], f32)
                for h in range(NH):
                    po = mmp.tile([P, NBLK], f32)
                    hs = slice(h * NBLK, (h + 1) * NBLK)
                    nc.tensor.matmul(
                        po[:], lhsT=vhi[:], rhs=uhi[:, hs], start=True, stop=False
                    )
                    nc.tensor.matmul(
                        po[:], lhsT=vhi[:], rhs=ulo[:, hs], start=False, stop=False
                    )
                    nc.tensor.matmul(
                        po[:], lhsT=vlo[:], rhs=uhi[:, hs], start=False, stop=True
                    )
                    if h % 2 == 0:
                        nc.scalar.copy(out=ot[:, hs], in_=po[:])
                    else:
                        nc.vector.tensor_copy(out=ot[:, hs], in_=po[:])
                nc.sync.dma_start(
                    out=out_dram[t * P : (t + 1) * P, :], in_=ot[:]
                )
    nc.finalize()
    return nc


def kernel(user_hiddens, item_hiddens, user_ids, item_ids, **_):
    user_hiddens = np.ascontiguousarray(user_hiddens, dtype=np.float32)
    item_hiddens = np.ascontiguousarray(item_hiddens, dtype=np.float32)
    user_ids = np.asarray(user_ids)
    item_ids = np.asarray(item_ids)

    if "nc" not in _cache:
        _cache["nc"] = _build()
    nc = _cache["nc"]

    in_maps = []
    for c in range(N_CORES):
        cu, ci = divmod(c, RI)
        uc = user_ids[cu * UC : (cu + 1) * UC]
        icd = item_ids[ci * IC : (ci + 1) * IC]
        # [P, T] transposed id layout: idx[p, t] = ids[t*128 + p]
        uids_t = np.ascontiguousarray(uc.astype(np.int32).reshape(UT, P).T)
        iids_t = np.ascontiguousarray(icd.astype(np.int32).reshape(IT, P).T)
        in_maps.append(
            {
                "user_table": user_hiddens,
                "item_table": item_hiddens,
                "uids": uids_t,
                "iids": iids_t,
            }
        )

    res = run_bass_kernel_spmd(nc, in_maps, list(range(N_CORES)))
    out = np.empty((BU, BI), dtype=np.float32)
    for c in range(N_CORES):
        cu, ci = divmod(c, RI)
        out[cu * UC : (cu + 1) * UC, ci * IC : (ci + 1) * IC] = res.results[c][
            "out"
        ].T
    return out
